# revision 2
# baseline (speedup 1.0000x reference)
"""BitNet MLP (ternary gate/up GEMM + silu*up + fwht + act-quant + down GEMM)
on 8 Trainium2 NeuronCores — v3.

Token-data-parallel across 8 cores (T=1024 tokens/core); a 4-float AllReduce
of |w| partial sums is the only cross-core traffic.

Key structure:
  - Host-transposed weights: every GEMM lhsT tile loads from HBM with the
    contraction dim on partitions — zero PE transposes for weights.
  - Ternarize / act-quant use the f32 +1.5*2^23 magic round (exact
    half-to-even rint, matching jnp.round bit-exactly); the integer results
    convert to fp16 exactly.
  - H64 butterflies of the FWHT run IN-PLACE on the fp16 intermediate,
    interleaved into the GEMM1 ic-loop as soon as their operand columns are
    ready (copy on scalar, add/sub on vector) — hidden under GEMM1's PE time.
  - After GEMM1: per t-block H128 via PE matmul (in-place through PSUM),
    per-token absmax via scalar Abs + in-place vector max tree + one gpsimd
    partition_all_reduce, then a 3-op quant chain.
  - Per-phase PSUM pools give GEMM1/GEMM2 all 8 banks (avoids PE stalls that
    let the HAM clock throttle).
  - GEMM2 emits the output transposed [H, T]; host un-transposes.
"""

import sys

sys.path.insert(0, "/opt/trn_rl_repo")

import numpy as np

import concourse.bass as bass
import concourse.mybir as mybir
import concourse.tile as tile
from concourse import bacc, bass_isa
from concourse.masks import make_identity

F32 = mybir.dt.float32
F16 = mybir.dt.float16
AX = mybir.AxisListType.X
OP = mybir.AluOpType
ACT = mybir.ActivationFunctionType

MAGIC = 12582912.0  # 1.5*2^23: (x + MAGIC) == rint(x) + MAGIC exactly in f32
QCLIP = 127.4375    # rint(min(t, QCLIP)) == min(rint(t), 127) exactly
EPS = 1e-5
TINY = 1e-20


def hadamard128():
    h = np.array([[1.0]], dtype=np.float32)
    while h.shape[0] < 128:
        h = np.block([[h, h], [h, -h]]).astype(np.float32)
    return h


def build_program(T, H, I, n_cores, debug=False):
    P = 128
    C = I // P
    HC = H // P
    NT = min(512, T)
    TTN = T // NT
    n_tb = T // P
    LC = int(np.log2(C))
    TB = 128              # fwht post-phase t-block
    NB = T // TB
    TBQ = 64              # quant sub-chunk
    WCH = 8               # h-chunks per weight-stage tile
    assert 2 ** LC == C and T % P == 0 and H % P == 0 and I % P == 0
    assert T % TB == 0 and TB % TBQ == 0 and HC % WCH == 0
    inv_sqrt_i = float(1.0 / np.sqrt(I))
    wcount = float(I) * float(H)
    SCW = min(1024, H, I)
    NXC = H // SCW

    nc = bacc.Bacc("TRN2", target_bir_lowering=False, num_devices=n_cores)

    x_d = nc.dram_tensor("x_s", [T, H], F32, kind="ExternalInput")
    wgT_d = nc.dram_tensor("wgT", [H, I], F32, kind="ExternalInput")
    wuT_d = nc.dram_tensor("wuT", [H, I], F32, kind="ExternalInput")
    wdT_d = nc.dram_tensor("wdT", [I, H], F32, kind="ExternalInput")
    wgs_d = nc.dram_tensor("wgs", [H // n_cores, I], F32, kind="ExternalInput")
    wus_d = nc.dram_tensor("wus", [H // n_cores, I], F32, kind="ExternalInput")
    wds_d = nc.dram_tensor("wds", [I // n_cores, H], F32, kind="ExternalInput")
    hm_d = nc.dram_tensor("hmat", [P, P], F32, kind="ExternalInput")
    out_d = nc.dram_tensor("out_s", [H, T], F32, kind="ExternalOutput")

    cc_in = nc.dram_tensor("cc_in", [1, 4], F32)
    cc_out = nc.dram_tensor("cc_out", [1, 4], F32, addr_space="Shared")
    if debug:
        dbg_yq = nc.dram_tensor("dbg_yq", [P, C * T], F16,
                                kind="ExternalOutput")
        dbg_amf = nc.dram_tensor("dbg_amf", [1, T], F32,
                                 kind="ExternalOutput")

    with tile.TileContext(nc) as tc:
        with (
            tc.tile_pool(name="consts", bufs=1) as consts,
            tc.tile_pool(name="rows", bufs=1) as rows,
            tc.tile_pool(name="ip", bufs=1) as ip,
        ):
            # ---------------- constants
            hmat_f = consts.tile([P, P], F32, tag="hmat_f")
            nc.sync.dma_start(hmat_f[:], hm_d.ap())
            hmat = consts.tile([P, P], F16, tag="hmat")
            nc.vector.tensor_copy(hmat[:], hmat_f[:])
            ident_f = consts.tile([P, P], F32, tag="ident_f")
            make_identity(nc, ident_f[:])
            ident_h = consts.tile([P, P], F16, tag="ident_h")
            nc.vector.tensor_copy(ident_h[:], ident_f[:])
            one_ap = nc.const_aps.tensor(1.0, (P, 1), F32)

            interm = ip.tile([P, C, T], F16, tag="interm")
            amf_row = rows.tile([1, T], F32, tag="amf_row")

            # ================= phase 0 + GEMM1 =================
            with tc.tile_pool(name="g1pool", bufs=1, named_scope="g1") as g1p:
                # ---- weight-scale pass (shard |w| sums + AllReduce)
                tot3 = rows.tile([P, 3], F32, tag="tot3")
                for mi, (src_d, rn, cn) in enumerate((
                        (wgs_d, H // n_cores, I),
                        (wus_d, H // n_cores, I),
                        (wds_d, I // n_cores, H))):
                    ntr, ntc = rn // P, cn // SCW
                    accm = rows.tile([P, ntr * ntc], F32, tag=f"accm{mi}")
                    for r in range(ntr):
                        for q in range(ntc):
                            st = g1p.tile([P, SCW], F32, tag="stage", bufs=2,
                                          name=f"sc{mi}_{r}_{q}")
                            nc.sync.dma_start(
                                st[:], src_d.ap()[r * P:(r + 1) * P,
                                                  q * SCW:(q + 1) * SCW])
                            nc.vector.tensor_reduce(
                                out=accm[:, r * ntc + q:r * ntc + q + 1],
                                in_=st[:], op=OP.add, axis=AX,
                                apply_absolute_value=True)
                    nc.vector.tensor_reduce(out=tot3[:, mi:mi + 1],
                                            in_=accm[:], op=OP.add, axis=AX)
                red3 = rows.tile([P, 3], F32, tag="red3")
                nc.gpsimd.partition_all_reduce(
                    red3[:], tot3[:], channels=P,
                    reduce_op=bass_isa.ReduceOp.add)
                ccin_sb = rows.tile([1, 4], F32, tag="ccin")
                nc.vector.memset(ccin_sb[:], 0.0)
                nc.vector.tensor_copy(ccin_sb[:, 0:3], red3[0:1, :])
                nc.sync.dma_start(cc_in.ap(), ccin_sb[:])
                nc.gpsimd.collective_compute(
                    "AllReduce", OP.add, ins=[cc_in.ap()], outs=[cc_out.ap()],
                    replica_groups=[list(range(n_cores))])
                sums_sb = rows.tile([1, 4], F32, tag="sums")
                nc.sync.dma_start(sums_sb[:], cc_out.ap())
                wm_row = rows.tile([1, 4], F32, tag="wm")
                nc.vector.tensor_scalar(wm_row[:], sums_sb[:], 1.0 / wcount,
                                        EPS, OP.mult, OP.max)
                ws_row = rows.tile([1, 4], F32, tag="ws")
                nc.vector.reciprocal(ws_row[:], wm_row[:])
                wsB = rows.tile([P, 4], F32, tag="wsB")
                nc.gpsimd.partition_broadcast(wsB[:], ws_row[:])

                # ---- x: act-quant + transpose into xqt [h_part, hc, t]
                xqt = g1p.tile([P, HC, T], F16, tag="xqt", bufs=1)
                am_row = rows.tile([1, T], F32, tag="am_row")
                with tc.tile_pool(name="psA", bufs=1, space="PSUM") as psA:
                    for tb in range(n_tb):
                        xts = []
                        am2 = rows.tile([P, NXC], F32, tag="am2", bufs=2,
                                        name=f"am2_{tb}")
                        for k in range(NXC):
                            xt = g1p.tile([P, SCW], F32, tag="stage", bufs=2,
                                          name=f"xt{tb}_{k}")
                            nc.sync.dma_start(
                                xt[:], x_d.ap()[tb * P:(tb + 1) * P,
                                                k * SCW:(k + 1) * SCW])
                            nc.vector.tensor_reduce(
                                out=am2[:, k:k + 1], in_=xt[:], op=OP.max,
                                axis=AX, apply_absolute_value=True)
                            xts.append(xt)
                        amc = rows.tile([P, 1], F32, tag="amc", bufs=2,
                                        name=f"amc{tb}")
                        nc.vector.tensor_reduce(out=amc[:], in_=am2[:],
                                                op=OP.max, axis=AX)
                        nc.vector.tensor_scalar(amc[:], amc[:], EPS, None,
                                                OP.max)
                        sx = rows.tile([P, 1], F32, tag="sx", bufs=2,
                                       name=f"sx{tb}")
                        nc.vector.reciprocal(sx[:], amc[:])
                        nc.vector.tensor_scalar(sx[:], sx[:], 128.0, None,
                                                OP.mult)
                        xq = g1p.tile([P, H], F16, tag="wq", bufs=2,
                                      name=f"xq{tb}")
                        for k in range(NXC):
                            nc.vector.tensor_scalar(xts[k][:], xts[k][:],
                                                    sx[:], QCLIP, OP.mult,
                                                    OP.min)
                            nc.vector.tensor_scalar(xts[k][:], xts[k][:],
                                                    MAGIC, None, OP.add)
                            nc.vector.tensor_scalar(
                                xq[:, k * SCW:(k + 1) * SCW], xts[k][:],
                                -MAGIC, None, OP.add)
                        for qg in range(HC // 4):
                            pt = psA.tile([P, 4 * P], F16, tag="tp", bufs=2,
                                          name=f"pt{tb}_{qg}")
                            for k in range(4):
                                hc = qg * 4 + k
                                nc.tensor.transpose(
                                    pt[:, k * P:(k + 1) * P],
                                    xq[:, hc * P:(hc + 1) * P], ident_h[:])
                            if qg % 2 == 0:
                                nc.scalar.copy(
                                    xqt[:, qg * 4:(qg + 1) * 4,
                                        tb * P:(tb + 1) * P], pt[:])
                            else:
                                nc.vector.tensor_copy(
                                    xqt[:, qg * 4:(qg + 1) * 4,
                                        tb * P:(tb + 1) * P], pt[:])
                        pr = psA.tile([P, P], F32, tag="tp", bufs=2,
                                      name=f"pr{tb}")
                        nc.tensor.transpose(pr[:1, :], amc[:], ident_f[:])
                        nc.scalar.copy(am_row[:, tb * P:(tb + 1) * P],
                                       pr[:1, :])

                # gate dequant row: am_row * wm_g / 128 (amf_row as scratch)
                nc.vector.tensor_scalar(amf_row[:], am_row[:],
                                        wm_row[0:1, 0:1], 1.0 / 128.0,
                                        OP.mult, OP.mult)
                row16 = g1p.tile([1, T], F16, tag="row16")
                nc.vector.tensor_copy(row16[:], amf_row[:])
                bcastG = g1p.tile([P, T], F16, tag="bcastG")
                nc.gpsimd.partition_broadcast(bcastG[:], row16[:])

                # ---- GEMM1 with interleaved in-place H64 butterflies
                def issue_bflies(k):
                    for s in range(LC):
                        bsz = 1 << (s + 1)
                        if (k + 1) % bsz != 0:
                            break
                        g0 = k + 1 - bsz
                        half = 1 << s
                        for ca in range(g0, g0 + half):
                            cb = ca + half
                            tmp = g1p.tile([P, 1, T], F16, tag="btmp",
                                           bufs=2, name=f"bt{s}_{ca}")
                            nc.scalar.copy(tmp[:], interm[:, ca:ca + 1, :])
                            nc.vector.tensor_tensor(
                                interm[:, ca:ca + 1, :],
                                interm[:, ca:ca + 1, :],
                                interm[:, cb:cb + 1, :], OP.add)
                            nc.vector.tensor_tensor(
                                interm[:, cb:cb + 1, :], tmp[:],
                                interm[:, cb:cb + 1, :], OP.subtract)

                with tc.tile_pool(name="psB", bufs=1, space="PSUM") as psB:
                    for ic in range(C):
                        psg = [psB.tile([P, NT], F32, tag="mm", bufs=8,
                                        name=f"psg{ic}_{_t}")
                               for _t in range(TTN)]
                        psu = [psB.tile([P, NT], F32, tag="mm", bufs=8,
                                        name=f"psu{ic}_{_t}")
                               for _t in range(TTN)]
                        for mi, wT_d in ((0, wgT_d), (1, wuT_d)):
                            pss = psg if mi == 0 else psu
                            wq = g1p.tile([P, HC, P], F16, tag="wq", bufs=2,
                                          name=f"wq{ic}_{mi}")
                            for ch in range(HC // WCH):
                                wst = g1p.tile([P, WCH, P], F32, tag="stage",
                                               bufs=2,
                                               name=f"wst{ic}_{mi}_{ch}")
                                nc.sync.dma_start(
                                    wst[:],
                                    wT_d.ap()[ch * WCH * P:(ch + 1) * WCH * P,
                                              ic * P:(ic + 1) * P]
                                    .rearrange("(hh p) i -> p hh i", p=P))
                                nc.scalar.activation(wst[:], wst[:], ACT.Relu,
                                                     bias=one_ap,
                                                     scale=wsB[:, mi:mi + 1])
                                wqs = wq[:, ch * WCH:(ch + 1) * WCH, :]
                                nc.vector.tensor_scalar(wst[:], wst[:], 2.0,
                                                        MAGIC, OP.min, OP.add)
                                nc.vector.tensor_scalar(wqs, wst[:],
                                                        -(MAGIC + 1.0), None,
                                                        OP.add)
                            for hc in range(HC):
                                for tt in range(TTN):
                                    nc.tensor.matmul(
                                        pss[tt][:], wq[:, hc, :],
                                        xqt[:, hc, tt * NT:(tt + 1) * NT],
                                        start=(hc == 0), stop=(hc == HC - 1))
                        for tt in range(TTN):
                            g1 = g1p.tile([P, NT], F32, tag="g1", bufs=2,
                                          name=f"g1_{ic}_{tt}")
                            nc.vector.tensor_tensor(
                                g1[:], psg[tt][:],
                                bcastG[:, tt * NT:(tt + 1) * NT], OP.mult)
                            nc.scalar.activation(g1[:], g1[:], ACT.Silu)
                            u1 = g1p.tile([P, NT], F16, tag="u1", bufs=1,
                                          name=f"u1_{ic}_{tt}")
                            nc.vector.tensor_scalar(u1[:], psu[tt][:],
                                                    1.0 / 1024.0, None,
                                                    OP.mult)
                            nc.vector.tensor_tensor(
                                interm[:, ic, tt * NT:(tt + 1) * NT], g1[:],
                                u1[:], OP.mult)
                        issue_bflies(ic)

            # ================= phase 2: H128 + act-quant ==============
            with (tc.tile_pool(name="fwpool", bufs=1, named_scope="fw")
                  as fwp,
                  tc.tile_pool(name="psC", bufs=1, space="PSUM") as psC):
                for blk in range(NB):
                    t0 = blk * TB
                    cols = slice(t0, t0 + TB)
                    iview = interm[:, :, cols]
                    for g in range(C * TB // NT):
                        ng = NT // TB  # c-chunks per matmul
                        sl = iview[:, g * ng:(g + 1) * ng, :]
                        pf = psC.tile([P, NT], F32, tag="tp", bufs=2,
                                      name=f"pf{blk}_{g}")
                        nc.tensor.matmul(pf[:], hmat[:], sl,
                                         start=True, stop=True)
                        nc.scalar.copy(sl, pf[:].rearrange(
                            "p (c t) -> p c t", c=ng))
                    za = fwp.tile([P, C, TB], F16, tag="za", bufs=2,
                                  name=f"za{blk}")
                    nc.scalar.activation(za[:], iview, ACT.Abs)
                    width = C
                    m1f = rows.tile([P, TB], F32, tag="m1f",
                                    name=f"m1f{blk}")
                    while width > 1:
                        half = width // 2
                        a = za[:, 0:half, :]
                        b = za[:, half:width, :]
                        if half == 1:
                            nc.vector.tensor_tensor(m1f[:, None, :], a, b,
                                                    OP.max)
                        else:
                            nc.vector.tensor_tensor(a, a, b, OP.max)
                        width = half
                    rc = rows.tile([P, TB], F32, tag="rc", name=f"rc{blk}")
                    nc.gpsimd.partition_all_reduce(
                        rc[:], m1f[:], channels=P,
                        reduce_op=bass_isa.ReduceOp.max)
                    nc.vector.tensor_scalar(rc[:], rc[:], TINY, None, OP.max)
                    nc.vector.tensor_copy(amf_row[:, cols], rc[0:1, :])
                    rec = rows.tile([P, TB], F32, tag="rec", name=f"rec{blk}")
                    nc.vector.reciprocal(rec[:], rc[:])
                    nc.vector.tensor_scalar(rec[:], rec[:], 128.0, None,
                                            OP.mult)
                    for sq in range(TB // TBQ):
                        scols = slice(t0 + sq * TBQ, t0 + (sq + 1) * TBQ)
                        isl = interm[:, :, scols]
                        q1 = fwp.tile([P, C, TBQ], F32, tag="q1", bufs=1,
                                      name=f"q1_{blk}_{sq}")
                        recb = rec[:, sq * TBQ:(sq + 1) * TBQ]
                        nc.vector.tensor_tensor(
                            q1[:], isl,
                            recb[:, None, :].to_broadcast((P, C, TBQ)),
                            OP.mult)
                        nc.vector.tensor_scalar(q1[:], q1[:], QCLIP, MAGIC,
                                                OP.min, OP.add)
                        nc.vector.tensor_scalar(isl, q1[:], -MAGIC, None,
                                                OP.add)

            if debug:
                nc.sync.dma_start(
                    dbg_yq.ap(), interm[:].rearrange("p c t -> p (c t)"))
                nc.sync.dma_start(dbg_amf.ap(), amf_row[:])

            # ================= phase 3: GEMM2 =================
            with (tc.tile_pool(name="g2pool", bufs=1, named_scope="g2")
                  as g2p,
                  tc.tile_pool(name="psD", bufs=1, space="PSUM") as psD):
                sf = rows.tile([1, 1], F32, tag="sf")
                nc.vector.tensor_tensor(sf[:], wm_row[:, 1:2], wm_row[:, 2:3],
                                        OP.mult)
                # x1024 compensates the 2^-10 psu scaling in the epilogue
                nc.vector.tensor_scalar(sf[:], sf[:],
                                        inv_sqrt_i * 1024.0 / (128.0 * 128.0),
                                        None, OP.mult)
                rowf = g2p.tile([1, T], F32, tag="rowf")
                nc.vector.tensor_scalar(rowf[:], am_row[:], sf[0:1, 0:1],
                                        None, OP.mult)
                nc.vector.tensor_tensor(rowf[:], rowf[:], amf_row[:],
                                        OP.mult)
                bcastF = rows.tile([P, T], F32, tag="bcastF")
                nc.gpsimd.partition_broadcast(bcastF[:], rowf[:])

                for hc in range(HC):
                    wdq = g2p.tile([P, C, P], F16, tag="wdq", bufs=2,
                                   name=f"wdq{hc}")
                    for ch in range(C // WCH):
                        wst = g2p.tile([P, WCH, P], F32, tag="wstage",
                                       bufs=2, name=f"wdst{hc}_{ch}")
                        nc.sync.dma_start(
                            wst[:],
                            wdT_d.ap()[ch * WCH * P:(ch + 1) * WCH * P,
                                       hc * P:(hc + 1) * P]
                            .rearrange("(cc p) x -> p cc x", p=P))
                        nc.scalar.activation(wst[:], wst[:], ACT.Relu,
                                             bias=one_ap, scale=wsB[:, 2:3])
                        wqs = wdq[:, ch * WCH:(ch + 1) * WCH, :]
                        nc.vector.tensor_scalar(wst[:], wst[:], 2.0, MAGIC,
                                                OP.min, OP.add)
                        nc.vector.tensor_scalar(wqs, wst[:], -(MAGIC + 1.0),
                                                None, OP.add)
                    pso = [psD.tile([P, NT], F32, tag="mm", bufs=8,
                                    name=f"pso{hc}_{_t}")
                           for _t in range(TTN)]
                    for c in range(C):
                        for tt in range(TTN):
                            nc.tensor.matmul(
                                pso[tt][:], wdq[:, c, :],
                                interm[:, c, tt * NT:(tt + 1) * NT],
                                start=(c == 0), stop=(c == C - 1))
                    for tt in range(TTN):
                        o1 = g2p.tile([P, NT], F32, tag="o1", bufs=2,
                                      name=f"o1_{hc}_{tt}")
                        nc.vector.tensor_tensor(
                            o1[:], pso[tt][:],
                            bcastF[:, tt * NT:(tt + 1) * NT], OP.mult)
                        nc.sync.dma_start(
                            out_d.ap()[hc * P:(hc + 1) * P,
                                       tt * NT:(tt + 1) * NT], o1[:])

    nc.compile()
    return nc


_PROG_CACHE = {}
_LAST_IN_MAPS = None


def kernel(x, w_gate, w_up, w_down):
    from concourse.bass_utils import run_bass_kernel_spmd

    B, S, H = x.shape
    I = w_gate.shape[0]
    n_cores = 8
    M = B * S
    T = M // n_cores

    key = (T, H, I, n_cores)
    if key not in _PROG_CACHE:
        _PROG_CACHE[key] = build_program(T, H, I, n_cores)
    nc = _PROG_CACHE[key]

    xf = np.ascontiguousarray(x.reshape(M, H).astype(np.float32))
    wgT = np.ascontiguousarray(w_gate.T.astype(np.float32))  # [H, I]
    wuT = np.ascontiguousarray(w_up.T.astype(np.float32))    # [H, I]
    wdT = np.ascontiguousarray(w_down.T.astype(np.float32))  # [I, H]
    hm = hadamard128()
    rs_g = H // n_cores
    rs_d = I // n_cores
    in_maps = []
    for c in range(n_cores):
        in_maps.append({
            "x_s": xf[c * T:(c + 1) * T],
            "wgT": wgT, "wuT": wuT, "wdT": wdT,
            "wgs": np.ascontiguousarray(wgT[c * rs_g:(c + 1) * rs_g]),
            "wus": np.ascontiguousarray(wuT[c * rs_g:(c + 1) * rs_g]),
            "wds": np.ascontiguousarray(wdT[c * rs_d:(c + 1) * rs_d]),
            "hmat": hm,
        })
    global _LAST_IN_MAPS
    _LAST_IN_MAPS = in_maps
    res = run_bass_kernel_spmd(nc, in_maps, list(range(n_cores)))
    out = np.concatenate(
        [res.results[c]["out_s"].T for c in range(n_cores)], axis=0)
    return out.reshape(B, S, H).astype(np.float32)


# revision 3
# speedup vs baseline: 1.0023x; 1.0023x over previous
"""BitNet MLP (ternary gate/up GEMM + silu*up + fwht + act-quant + down GEMM)
on 8 Trainium2 NeuronCores — v3.

Token-data-parallel across 8 cores (T=1024 tokens/core); a 4-float AllReduce
of |w| partial sums is the only cross-core traffic.

Key structure:
  - Host-transposed weights: every GEMM lhsT tile loads from HBM with the
    contraction dim on partitions — zero PE transposes for weights.
  - Ternarize / act-quant use the f32 +1.5*2^23 magic round (exact
    half-to-even rint, matching jnp.round bit-exactly); the integer results
    convert to fp16 exactly.
  - H64 butterflies of the FWHT run IN-PLACE on the fp16 intermediate,
    interleaved into the GEMM1 ic-loop as soon as their operand columns are
    ready (copy on scalar, add/sub on vector) — hidden under GEMM1's PE time.
  - After GEMM1: per t-block H128 via PE matmul (in-place through PSUM),
    per-token absmax via scalar Abs + in-place vector max tree + one gpsimd
    partition_all_reduce, then a 3-op quant chain.
  - Per-phase PSUM pools give GEMM1/GEMM2 all 8 banks (avoids PE stalls that
    let the HAM clock throttle).
  - GEMM2 emits the output transposed [H, T]; host un-transposes.
"""

import sys

sys.path.insert(0, "/opt/trn_rl_repo")

import numpy as np

import concourse.bass as bass
import concourse.mybir as mybir
import concourse.tile as tile
from concourse import bacc, bass_isa
from concourse.masks import make_identity

F32 = mybir.dt.float32
F16 = mybir.dt.float16
AX = mybir.AxisListType.X
OP = mybir.AluOpType
ACT = mybir.ActivationFunctionType

MAGIC = 12582912.0  # 1.5*2^23: (x + MAGIC) == rint(x) + MAGIC exactly in f32
QCLIP = 127.4375    # rint(min(t, QCLIP)) == min(rint(t), 127) exactly
EPS = 1e-5
TINY = 1e-20


def hadamard128():
    h = np.array([[1.0]], dtype=np.float32)
    while h.shape[0] < 128:
        h = np.block([[h, h], [h, -h]]).astype(np.float32)
    return h


def build_program(T, H, I, n_cores, debug=False):
    P = 128
    C = I // P
    HC = H // P
    NT = min(512, T)
    TTN = T // NT
    n_tb = T // P
    LC = int(np.log2(C))
    TB = 128              # fwht post-phase t-block
    NB = T // TB
    TBQ = 64              # quant sub-chunk
    WCH = 8               # h-chunks per weight-stage tile
    assert 2 ** LC == C and T % P == 0 and H % P == 0 and I % P == 0
    assert T % TB == 0 and TB % TBQ == 0 and HC % WCH == 0
    inv_sqrt_i = float(1.0 / np.sqrt(I))
    wcount = float(I) * float(H)
    SCW = min(1024, H, I)
    NXC = H // SCW

    nc = bacc.Bacc("TRN2", target_bir_lowering=False, num_devices=n_cores)

    x_d = nc.dram_tensor("x_s", [T, H], F32, kind="ExternalInput")
    wgT_d = nc.dram_tensor("wgT", [H, I], F32, kind="ExternalInput")
    wuT_d = nc.dram_tensor("wuT", [H, I], F32, kind="ExternalInput")
    wdT_d = nc.dram_tensor("wdT", [I, H], F32, kind="ExternalInput")
    wgs_d = nc.dram_tensor("wgs", [H // n_cores, I], F32, kind="ExternalInput")
    wus_d = nc.dram_tensor("wus", [H // n_cores, I], F32, kind="ExternalInput")
    wds_d = nc.dram_tensor("wds", [I // n_cores, H], F32, kind="ExternalInput")
    hm_d = nc.dram_tensor("hmat", [P, P], F32, kind="ExternalInput")
    out_d = nc.dram_tensor("out_s", [H, T], F32, kind="ExternalOutput")

    cc_ins = [nc.dram_tensor(f"cc_in{m}", [1, 4], F32) for m in range(3)]
    cc_outs = [nc.dram_tensor(f"cc_out{m}", [1, 4], F32, addr_space="Shared")
               for m in range(3)]
    if debug:
        dbg_yq = nc.dram_tensor("dbg_yq", [P, C * T], F16,
                                kind="ExternalOutput")
        dbg_amf = nc.dram_tensor("dbg_amf", [1, T], F32,
                                 kind="ExternalOutput")

    with tile.TileContext(nc) as tc:
        with (
            tc.tile_pool(name="consts", bufs=1) as consts,
            tc.tile_pool(name="rows", bufs=1) as rows,
            tc.tile_pool(name="ip", bufs=1) as ip,
        ):
            # ---------------- constants
            hmat_f = consts.tile([P, P], F32, tag="hmat_f")
            nc.sync.dma_start(hmat_f[:], hm_d.ap())
            hmat = consts.tile([P, P], F16, tag="hmat")
            nc.vector.tensor_copy(hmat[:], hmat_f[:])
            ident_f = consts.tile([P, P], F32, tag="ident_f")
            make_identity(nc, ident_f[:])
            ident_h = consts.tile([P, P], F16, tag="ident_h")
            nc.vector.tensor_copy(ident_h[:], ident_f[:])
            one_ap = nc.const_aps.tensor(1.0, (P, 1), F32)

            interm = ip.tile([P, C, T], F16, tag="interm")
            amf_row = rows.tile([1, T], F32, tag="amf_row")

            # ================= phase 0 + GEMM1 =================
            with tc.tile_pool(name="g1pool", bufs=1, named_scope="g1") as g1p:
                # ---- weight-scale pass: per-matrix shard |w| sums, each
                # AllReduced as soon as its sums finish (gate first, so
                # GEMM1's ternarize can start earliest)
                wm_row = rows.tile([1, 4], F32, tag="wm")
                wsB = rows.tile([P, 4], F32, tag="wsB")
                for mi, (src_d, rn, cn) in enumerate((
                        (wgs_d, H // n_cores, I),
                        (wus_d, H // n_cores, I),
                        (wds_d, I // n_cores, H))):
                    SCW2 = 512
                    ntr, ntc = rn // P, cn // SCW2
                    accm = rows.tile([P, ntr * ntc], F32, tag=f"accm{mi}")
                    for r in range(ntr):
                        for q in range(ntc):
                            st = g1p.tile([P, SCW2], F32, tag="scstage",
                                          bufs=2, name=f"sc{mi}_{r}_{q}")
                            nc.sync.dma_start(
                                st[:], src_d.ap()[r * P:(r + 1) * P,
                                                  q * SCW2:(q + 1) * SCW2])
                            nc.vector.tensor_reduce(
                                out=accm[:, r * ntc + q:r * ntc + q + 1],
                                in_=st[:], op=OP.add, axis=AX,
                                apply_absolute_value=True)
                    totm = rows.tile([P, 1], F32, tag=f"tot{mi}")
                    nc.vector.tensor_reduce(out=totm[:], in_=accm[:],
                                            op=OP.add, axis=AX)
                    redm = rows.tile([P, 1], F32, tag=f"red{mi}")
                    nc.gpsimd.partition_all_reduce(
                        redm[:], totm[:], channels=P,
                        reduce_op=bass_isa.ReduceOp.add)
                    ccin_sb = rows.tile([1, 4], F32, tag=f"ccin{mi}")
                    nc.vector.memset(ccin_sb[:], 0.0)
                    nc.vector.tensor_copy(ccin_sb[:, 0:1], redm[0:1, :])
                    nc.sync.dma_start(cc_ins[mi].ap(), ccin_sb[:])
                    nc.gpsimd.collective_compute(
                        "AllReduce", OP.add, ins=[cc_ins[mi].ap()],
                        outs=[cc_outs[mi].ap()],
                        replica_groups=[list(range(n_cores))])
                    sums_sb = rows.tile([1, 4], F32, tag=f"sums{mi}")
                    nc.sync.dma_start(sums_sb[:], cc_outs[mi].ap())
                    nc.vector.tensor_scalar(wm_row[:, mi:mi + 1],
                                            sums_sb[:, 0:1], 1.0 / wcount,
                                            EPS, OP.mult, OP.max)
                    wsm = rows.tile([1, 1], F32, tag=f"ws{mi}")
                    nc.vector.reciprocal(wsm[:], wm_row[:, mi:mi + 1])
                    nc.gpsimd.partition_broadcast(wsB[:, mi:mi + 1], wsm[:])

                # ---- x: act-quant + transpose into xqt [h_part, hc, t]
                xqt = g1p.tile([P, HC, T], F16, tag="xqt", bufs=1)
                am_row = rows.tile([1, T], F32, tag="am_row")
                with tc.tile_pool(name="psA", bufs=1, space="PSUM") as psA:
                    for tb in range(n_tb):
                        xts = []
                        am2 = rows.tile([P, NXC], F32, tag="am2", bufs=2,
                                        name=f"am2_{tb}")
                        for k in range(NXC):
                            xt = g1p.tile([P, SCW], F32, tag="stage", bufs=2,
                                          name=f"xt{tb}_{k}")
                            nc.sync.dma_start(
                                xt[:], x_d.ap()[tb * P:(tb + 1) * P,
                                                k * SCW:(k + 1) * SCW])
                            nc.vector.tensor_reduce(
                                out=am2[:, k:k + 1], in_=xt[:], op=OP.max,
                                axis=AX, apply_absolute_value=True)
                            xts.append(xt)
                        amc = rows.tile([P, 1], F32, tag="amc", bufs=2,
                                        name=f"amc{tb}")
                        nc.vector.tensor_reduce(out=amc[:], in_=am2[:],
                                                op=OP.max, axis=AX)
                        nc.vector.tensor_scalar(amc[:], amc[:], EPS, None,
                                                OP.max)
                        sx = rows.tile([P, 1], F32, tag="sx", bufs=2,
                                       name=f"sx{tb}")
                        nc.vector.reciprocal(sx[:], amc[:])
                        nc.vector.tensor_scalar(sx[:], sx[:], 128.0, None,
                                                OP.mult)
                        xq = g1p.tile([P, H], F16, tag="wq", bufs=2,
                                      name=f"xq{tb}")
                        for k in range(NXC):
                            nc.vector.tensor_scalar(xts[k][:], xts[k][:],
                                                    sx[:], QCLIP, OP.mult,
                                                    OP.min)
                            nc.vector.tensor_scalar(xts[k][:], xts[k][:],
                                                    MAGIC, None, OP.add)
                            nc.vector.tensor_scalar(
                                xq[:, k * SCW:(k + 1) * SCW], xts[k][:],
                                -MAGIC, None, OP.add)
                        for qg in range(HC // 4):
                            pt = psA.tile([P, 4 * P], F16, tag="tp", bufs=2,
                                          name=f"pt{tb}_{qg}")
                            for k in range(4):
                                hc = qg * 4 + k
                                nc.tensor.transpose(
                                    pt[:, k * P:(k + 1) * P],
                                    xq[:, hc * P:(hc + 1) * P], ident_h[:])
                            if qg % 2 == 0:
                                nc.scalar.copy(
                                    xqt[:, qg * 4:(qg + 1) * 4,
                                        tb * P:(tb + 1) * P], pt[:])
                            else:
                                nc.vector.tensor_copy(
                                    xqt[:, qg * 4:(qg + 1) * 4,
                                        tb * P:(tb + 1) * P], pt[:])
                        pr = psA.tile([P, P], F32, tag="tp", bufs=2,
                                      name=f"pr{tb}")
                        nc.tensor.transpose(pr[:1, :], amc[:], ident_f[:])
                        nc.scalar.copy(am_row[:, tb * P:(tb + 1) * P],
                                       pr[:1, :])

                # gate dequant row: am_row * wm_g / 128 (amf_row as scratch)
                nc.vector.tensor_scalar(amf_row[:], am_row[:],
                                        wm_row[0:1, 0:1], 1.0 / 128.0,
                                        OP.mult, OP.mult)
                row16 = g1p.tile([1, T], F16, tag="row16")
                nc.vector.tensor_copy(row16[:], amf_row[:])
                bcastG = g1p.tile([P, T], F16, tag="bcastG")
                nc.gpsimd.partition_broadcast(bcastG[:], row16[:])

                # ---- GEMM1 with interleaved in-place H64 butterflies
                def issue_bflies(k):
                    for s in range(LC):
                        bsz = 1 << (s + 1)
                        if (k + 1) % bsz != 0:
                            break
                        g0 = k + 1 - bsz
                        half = 1 << s
                        for ca in range(g0, g0 + half):
                            cb = ca + half
                            tmp = g1p.tile([P, 1, T], F16, tag="btmp",
                                           bufs=1, name=f"bt{s}_{ca}")
                            nc.scalar.copy(tmp[:], interm[:, ca:ca + 1, :])
                            nc.vector.tensor_tensor(
                                interm[:, ca:ca + 1, :],
                                interm[:, ca:ca + 1, :],
                                interm[:, cb:cb + 1, :], OP.add)
                            nc.vector.tensor_tensor(
                                interm[:, cb:cb + 1, :], tmp[:],
                                interm[:, cb:cb + 1, :], OP.subtract)

                with tc.tile_pool(name="psB", bufs=1, space="PSUM") as psB:
                    for ic in range(C):
                        psg = [psB.tile([P, NT], F32, tag="mm", bufs=8,
                                        name=f"psg{ic}_{_t}")
                               for _t in range(TTN)]
                        psu = [psB.tile([P, NT], F32, tag="mm", bufs=8,
                                        name=f"psu{ic}_{_t}")
                               for _t in range(TTN)]
                        for mi, wT_d in ((0, wgT_d), (1, wuT_d)):
                            pss = psg if mi == 0 else psu
                            wq = g1p.tile([P, HC, P], F16, tag="wq", bufs=2,
                                          name=f"wq{ic}_{mi}")
                            for ch in range(HC // WCH):
                                wst = g1p.tile([P, WCH, P], F32, tag="stage",
                                               bufs=2,
                                               name=f"wst{ic}_{mi}_{ch}")
                                nc.sync.dma_start(
                                    wst[:],
                                    wT_d.ap()[ch * WCH * P:(ch + 1) * WCH * P,
                                              ic * P:(ic + 1) * P]
                                    .rearrange("(hh p) i -> p hh i", p=P))
                                nc.scalar.activation(wst[:], wst[:], ACT.Relu,
                                                     bias=one_ap,
                                                     scale=wsB[:, mi:mi + 1])
                                wqs = wq[:, ch * WCH:(ch + 1) * WCH, :]
                                nc.vector.tensor_scalar(wst[:], wst[:], 2.0,
                                                        MAGIC, OP.min, OP.add)
                                nc.vector.tensor_scalar(wqs, wst[:],
                                                        -(MAGIC + 1.0), None,
                                                        OP.add)
                            for hc in range(HC):
                                for tt in range(TTN):
                                    nc.tensor.matmul(
                                        pss[tt][:], wq[:, hc, :],
                                        xqt[:, hc, tt * NT:(tt + 1) * NT],
                                        start=(hc == 0), stop=(hc == HC - 1))
                        for tt in range(TTN):
                            g1 = g1p.tile([P, NT], F32, tag="g1", bufs=1,
                                          name=f"g1_{ic}_{tt}")
                            nc.vector.tensor_tensor(
                                g1[:], psg[tt][:],
                                bcastG[:, tt * NT:(tt + 1) * NT], OP.mult)
                            nc.scalar.activation(g1[:], g1[:], ACT.Silu)
                            u1 = g1p.tile([P, NT], F16, tag="u1", bufs=1,
                                          name=f"u1_{ic}_{tt}")
                            nc.vector.tensor_scalar(u1[:], psu[tt][:],
                                                    1.0 / 1024.0, None,
                                                    OP.mult)
                            nc.vector.tensor_tensor(
                                interm[:, ic, tt * NT:(tt + 1) * NT], g1[:],
                                u1[:], OP.mult)
                        issue_bflies(ic)

            # ================= phase 2: H128 + act-quant ==============
            with (tc.tile_pool(name="fwpool", bufs=1, named_scope="fw")
                  as fwp,
                  tc.tile_pool(name="psC", bufs=1, space="PSUM") as psC):
                for blk in range(NB):
                    t0 = blk * TB
                    cols = slice(t0, t0 + TB)
                    iview = interm[:, :, cols]
                    for g in range(C * TB // NT):
                        ng = NT // TB  # c-chunks per matmul
                        sl = iview[:, g * ng:(g + 1) * ng, :]
                        pf = psC.tile([P, NT], F32, tag="tp", bufs=2,
                                      name=f"pf{blk}_{g}")
                        nc.tensor.matmul(pf[:], hmat[:], sl,
                                         start=True, stop=True)
                        pfv = pf[:].rearrange("p (c t) -> p c t", c=ng)
                        if g % 2 == 0:
                            nc.scalar.copy(sl, pfv)
                        else:
                            nc.vector.tensor_copy(sl, pfv)
                    za = fwp.tile([P, C, TB], F16, tag="za", bufs=2,
                                  name=f"za{blk}")
                    nc.scalar.activation(za[:], iview, ACT.Abs)
                    width = C
                    m1f = rows.tile([P, TB], F32, tag="m1f",
                                    name=f"m1f{blk}")
                    while width > 1:
                        half = width // 2
                        a = za[:, 0:half, :]
                        b = za[:, half:width, :]
                        if half == 1:
                            nc.vector.tensor_tensor(m1f[:, None, :], a, b,
                                                    OP.max)
                        else:
                            nc.vector.tensor_tensor(a, a, b, OP.max)
                        width = half
                    rc = rows.tile([P, TB], F32, tag="rc", name=f"rc{blk}")
                    nc.gpsimd.partition_all_reduce(
                        rc[:], m1f[:], channels=P,
                        reduce_op=bass_isa.ReduceOp.max)
                    nc.vector.tensor_scalar(rc[:], rc[:], TINY, None, OP.max)
                    nc.vector.tensor_copy(amf_row[:, cols], rc[0:1, :])
                    rec = rows.tile([P, TB], F32, tag="rec", name=f"rec{blk}")
                    nc.vector.reciprocal(rec[:], rc[:])
                    nc.vector.tensor_scalar(rec[:], rec[:], 128.0, None,
                                            OP.mult)
                    for sq in range(TB // TBQ):
                        scols = slice(t0 + sq * TBQ, t0 + (sq + 1) * TBQ)
                        isl = interm[:, :, scols]
                        q1 = fwp.tile([P, C, TBQ], F32, tag="q1", bufs=1,
                                      name=f"q1_{blk}_{sq}")
                        recb = rec[:, sq * TBQ:(sq + 1) * TBQ]
                        nc.vector.tensor_tensor(
                            q1[:], isl,
                            recb[:, None, :].to_broadcast((P, C, TBQ)),
                            OP.mult)
                        nc.vector.tensor_scalar(q1[:], q1[:], QCLIP, MAGIC,
                                                OP.min, OP.add)
                        nc.vector.tensor_scalar(isl, q1[:], -MAGIC, None,
                                                OP.add)

            if debug:
                nc.sync.dma_start(
                    dbg_yq.ap(), interm[:].rearrange("p c t -> p (c t)"))
                nc.sync.dma_start(dbg_amf.ap(), amf_row[:])

            # ================= phase 3: GEMM2 =================
            with (tc.tile_pool(name="g2pool", bufs=1, named_scope="g2")
                  as g2p,
                  tc.tile_pool(name="psD", bufs=1, space="PSUM") as psD):
                sf = rows.tile([1, 1], F32, tag="sf")
                nc.vector.tensor_tensor(sf[:], wm_row[:, 1:2], wm_row[:, 2:3],
                                        OP.mult)
                # x1024 compensates the 2^-10 psu scaling in the epilogue
                nc.vector.tensor_scalar(sf[:], sf[:],
                                        inv_sqrt_i * 1024.0 / (128.0 * 128.0),
                                        None, OP.mult)
                rowf = g2p.tile([1, T], F32, tag="rowf")
                nc.vector.tensor_scalar(rowf[:], am_row[:], sf[0:1, 0:1],
                                        None, OP.mult)
                nc.vector.tensor_tensor(rowf[:], rowf[:], amf_row[:],
                                        OP.mult)
                bcastF = rows.tile([P, T], F32, tag="bcastF")
                nc.gpsimd.partition_broadcast(bcastF[:], rowf[:])

                for hc in range(HC):
                    wdq = g2p.tile([P, C, P], F16, tag="wdq", bufs=2,
                                   name=f"wdq{hc}")
                    for ch in range(C // WCH):
                        wst = g2p.tile([P, WCH, P], F32, tag="wstage",
                                       bufs=2, name=f"wdst{hc}_{ch}")
                        nc.sync.dma_start(
                            wst[:],
                            wdT_d.ap()[ch * WCH * P:(ch + 1) * WCH * P,
                                       hc * P:(hc + 1) * P]
                            .rearrange("(cc p) x -> p cc x", p=P))
                        nc.scalar.activation(wst[:], wst[:], ACT.Relu,
                                             bias=one_ap, scale=wsB[:, 2:3])
                        wqs = wdq[:, ch * WCH:(ch + 1) * WCH, :]
                        nc.vector.tensor_scalar(wst[:], wst[:], 2.0, MAGIC,
                                                OP.min, OP.add)
                        nc.vector.tensor_scalar(wqs, wst[:], -(MAGIC + 1.0),
                                                None, OP.add)
                    pso = [psD.tile([P, NT], F32, tag="mm", bufs=8,
                                    name=f"pso{hc}_{_t}")
                           for _t in range(TTN)]
                    for c in range(C):
                        for tt in range(TTN):
                            nc.tensor.matmul(
                                pso[tt][:], wdq[:, c, :],
                                interm[:, c, tt * NT:(tt + 1) * NT],
                                start=(c == 0), stop=(c == C - 1))
                    for tt in range(TTN):
                        o1 = g2p.tile([P, NT], F32, tag="o1", bufs=2,
                                      name=f"o1_{hc}_{tt}")
                        nc.vector.tensor_tensor(
                            o1[:], pso[tt][:],
                            bcastF[:, tt * NT:(tt + 1) * NT], OP.mult)
                        nc.sync.dma_start(
                            out_d.ap()[hc * P:(hc + 1) * P,
                                       tt * NT:(tt + 1) * NT], o1[:])

    nc.compile()
    return nc


_PROG_CACHE = {}
_LAST_IN_MAPS = None


def kernel(x, w_gate, w_up, w_down):
    from concourse.bass_utils import run_bass_kernel_spmd

    B, S, H = x.shape
    I = w_gate.shape[0]
    n_cores = 8
    M = B * S
    T = M // n_cores

    key = (T, H, I, n_cores)
    if key not in _PROG_CACHE:
        _PROG_CACHE[key] = build_program(T, H, I, n_cores)
    nc = _PROG_CACHE[key]

    xf = np.ascontiguousarray(x.reshape(M, H).astype(np.float32))
    wgT = np.ascontiguousarray(w_gate.T.astype(np.float32))  # [H, I]
    wuT = np.ascontiguousarray(w_up.T.astype(np.float32))    # [H, I]
    wdT = np.ascontiguousarray(w_down.T.astype(np.float32))  # [I, H]
    hm = hadamard128()
    rs_g = H // n_cores
    rs_d = I // n_cores
    in_maps = []
    for c in range(n_cores):
        in_maps.append({
            "x_s": xf[c * T:(c + 1) * T],
            "wgT": wgT, "wuT": wuT, "wdT": wdT,
            "wgs": np.ascontiguousarray(wgT[c * rs_g:(c + 1) * rs_g]),
            "wus": np.ascontiguousarray(wuT[c * rs_g:(c + 1) * rs_g]),
            "wds": np.ascontiguousarray(wdT[c * rs_d:(c + 1) * rs_d]),
            "hmat": hm,
        })
    global _LAST_IN_MAPS
    _LAST_IN_MAPS = in_maps
    res = run_bass_kernel_spmd(nc, in_maps, list(range(n_cores)))
    out = np.concatenate(
        [res.results[c]["out_s"].T for c in range(n_cores)], axis=0)
    return out.reshape(B, S, H).astype(np.float32)
